# revision 29
# baseline (speedup 1.0000x reference)
"""Trainium2 Bass/Tile kernel for a pre-norm causal decoder block.

Math (matches the jax reference):
    h   = LN1(x) * g1 + beta1
    q,k,v = per-head projections of h (D_HEAD=21, 6 heads)
    sT  = (k @ q^T) / sqrt(21) + causal mask        (scores, transposed)
    e   = exp(sT)                                   (no max-subtraction; scores are tiny)
    o   = (e^T @ [v | 1]) -> per-(t,head) denominator in the appended column
    att = (o / denom) @ Wo + bo
    x1  = x + att
    out = x1 + relu(LN2(x1)*g2+beta2 @ W1 + b1) @ W2 + b2

Sharding: pure data parallelism, batch 512 -> 64 per core across 8 cores.

Layout strategy (per core):
  - tokens T=128 occupy SBUF partitions for LN/residual phases
  - LN mean/var via bn_stats+bn_aggr; rstd = Exp(-0.5*Ln(var+eps)) so every
    activation used (Ln/Exp/Identity/Copy/Relu) lives in ONE act table set
    (natural_log_exp_and_others); a Bacc subclass steers the act-table pass
    there so the table is loaded exactly once
  - LN centering is a single Act instruction per batch:
    Identity(scale=rstd, bias=-mean*rstd)
  - [t,d]->[d,t] transposes (hh, o, h2) run on the PE, 4 batches into one
    PSUM bank, drained by a single wide DVE/Act copy
  - qT/kT are stored head-padded to 32 partitions (pair-packed into 3x64)
    so score matmuls are K=32 row-tiles; adjacent heads alternate PSUM banks
    (same-bank concurrent PE writes crash the device)
  - scores are computed transposed (sT[s,t]) so the softmax denominator is
    a matmul-accumulated ones-column and no attention transpose is needed
  - causal mask is added in-PSUM via an identity matmul (values -30 => exp ~ 1e-13)
  - all matmul operands bf16, PSUM accumulation fp32, LN/softmax arithmetic fp32
"""

import os
import numpy as np
import ml_dtypes

from contextlib import ExitStack

import bass_rust
import concourse.bass as bass
import concourse.bacc as bacc
import concourse.tile as tile
from concourse import mybir
from concourse.bass_utils import run_bass_kernel_spmd
from concourse.hw_specs import get_activation_tables

BF = mybir.dt.bfloat16
F32 = mybir.dt.float32
NPBF = ml_dtypes.bfloat16

B, T, D = 512, 128, 128
NH, DH = 6, 21
DC = NH * DH  # 126
DFF = 512
NCORES = 8
BPC = B // NCORES  # 64 batches per core
G = 4              # batches per group (free-dim batching of qkv projections)
EPS = 1e-5
MASK_NEG = -30.0
SM_SCALE = 1.0 / np.sqrt(np.float32(DH))

AF = mybir.ActivationFunctionType
ALU = mybir.AluOpType
AX = mybir.AxisListType

# All activation funcs this kernel emits; they all live together in the
# natural_log_exp_and_others act-table set.
_ACT_SET = "natural_log_exp_and_others"
_OUR_FUNCS = {AF.Exp, AF.Ln, AF.Identity, AF.Copy, AF.Relu}


class _Bacc(bacc.Bacc):
    """Bacc whose act-table pass is steered to a single table set.

    The stock pass picks, per activation, some table containing its
    function; Exp resolves to `exp_and_others` while Ln resolves to
    `natural_log`, so interleaved Exp/Ln activations reload the table
    (1283 ns each).  Stripping our functions from every candidate except
    `natural_log_exp_and_others` (which truly contains all of them)
    forces one choice -> the table loads once.  Table ids stay positional
    so the runtime mapping is unchanged.
    """

    def insert_act_table_loads(self):
        has_activation = any(
            isinstance(i, mybir.InstActivation)
            for b in self.main_func.blocks
            for i in b.instructions
        )
        if not has_activation:
            return
        tables = []
        for name, funcs in get_activation_tables(self.m.arch).items():
            if name != _ACT_SET:
                funcs = funcs - _OUR_FUNCS
            tables.append((name, funcs))
        bass_rust.insert_act_table_loads(self, tables)


def _bf(a):
    return np.ascontiguousarray(np.asarray(a, dtype=np.float32)).astype(NPBF)


def _prep_weights(Wq, Wk, Wv, Wo, bo, W1, b1, W2, b2, g1, beta1, g2, beta2):
    """Host-side folding/packing. Returns dict of named arrays + flags."""
    Wq = np.asarray(Wq, np.float64)
    Wk = np.asarray(Wk, np.float64)
    Wv = np.asarray(Wv, np.float64)
    g1 = np.asarray(g1, np.float64)
    g2 = np.asarray(g2, np.float64)
    beta1 = np.asarray(beta1, np.float64)
    beta2 = np.asarray(beta2, np.float64)
    W1 = np.asarray(W1, np.float64)

    # fold g1 into the qkv projections, 1/sqrt(DH) into Wq
    Wq_f = g1[None, :, None] * Wq * SM_SCALE   # [h, d, e]
    Wk_f = g1[None, :, None] * Wk
    Wv_f = g1[None, :, None] * Wv

    # pair-packed q/k: tensor i holds heads 2i (rows 0..20) and 2i+1 (rows 32..52)
    wq_p = np.zeros((3, D, 64), np.float64)
    wk_p = np.zeros((3, D, 64), np.float64)
    for h in range(NH):
        i, off = h // 2, 32 * (h % 2)
        wq_p[i, :, off:off + DH] = Wq_f[h]
        wk_p[i, :, off:off + DH] = Wk_f[h]
    wv = np.concatenate([Wv_f[h] for h in range(NH)], axis=1)  # [128, 126]

    # beta1 contributions (rank-1 into qT/kT/v)
    qb = np.einsum("d,hde->he", beta1, Wq) * SM_SCALE   # [6, 21]
    kb = np.einsum("d,hde->he", beta1, Wk)
    vb = np.einsum("d,hde->he", beta1, Wv)
    qb_p = np.zeros((64, 3), np.float64)
    kb_p = np.zeros((64, 3), np.float64)
    for h in range(NH):
        i, off = h // 2, 32 * (h % 2)
        qb_p[off:off + DH, i] = qb[h]
        kb_p[off:off + DH, i] = kb[h]
    vb_r = vb.reshape(1, DC)

    w1 = g2[:, None] * W1                     # [128, 512]
    b1_eff = np.asarray(b1, np.float64) + beta2 @ W1   # [512]
    w2 = np.asarray(W2, np.float64).reshape(4, 128, D).transpose(1, 0, 2)  # [128,4,128]

    # multiplicative causal mask (post-exp): 1 where s <= t else 0, for all
    # six 128-col head blocks of eT
    mask1 = np.where(np.arange(T)[:, None] <= np.arange(T)[None, :], 1.0, 0.0)
    mask6 = np.tile(mask1, (1, 6))            # [T, 768]

    out = {
        "wq_p": _bf(wq_p), "wk_p": _bf(wk_p),
        "wv": _bf(wv),
        "wo": _bf(Wo), "w1": _bf(w1), "w2": _bf(w2),
        "mask6": _bf(mask6),
        "ident": _bf(np.eye(128)),
        "qb_p": np.asarray(qb_p, np.float32),
        "kb_p": np.asarray(kb_p, np.float32),
        "vb_r": np.asarray(vb_r, np.float32),
        "bo_r": np.asarray(bo, np.float32).reshape(1, D),
        "b2_r": np.asarray(b2, np.float32).reshape(1, D),
        "b1e": np.ascontiguousarray(
            np.asarray(b1_eff, np.float64).reshape(4, 128).T, dtype=np.float32
        ),  # [128, 4] per-partition relu bias per chunk
    }
    flags = {
        "qkv_bias": bool(np.any(beta1 != 0.0)),
        "bo": bool(np.any(np.asarray(bo) != 0.0)),
        "b2": bool(np.any(np.asarray(b2) != 0.0)),
        "b1": bool(np.any(out["b1e"] != 0.0)),
    }
    return out, flags


def _emit(ctx, tc, aps, flags, bpc):
    nc = tc.nc
    x_ap = aps["x"]
    y_ap = aps["y"]

    singles = ctx.enter_context(tc.tile_pool(name="singles", bufs=1))
    sb_g = ctx.enter_context(tc.tile_pool(name="sb_g", bufs=4))
    sb_b = ctx.enter_context(tc.tile_pool(name="sb_b", bufs=6))
    sb_s = ctx.enter_context(tc.tile_pool(name="sb_s", bufs=12))
    psA = ctx.enter_context(tc.tile_pool(name="psA", bufs=2, space="PSUM"))
    pss = ctx.enter_context(tc.tile_pool(name="pss", bufs=2, space="PSUM"))
    pso = ctx.enter_context(tc.tile_pool(name="pso", bufs=2, space="PSUM"))
    psf = ctx.enter_context(tc.tile_pool(name="psf", bufs=2, space="PSUM"))

    # ---- resident constants -------------------------------------------------
    def load_const(name, shape, dtype=BF):
        t = singles.tile(list(shape), dtype, tag=name)
        nc.sync.dma_start(out=t[:], in_=aps[name])
        return t

    wq_p = singles.tile([D, 3, 64], BF, tag="wq_p")
    nc.sync.dma_start(out=wq_p[:], in_=aps["wq_p"].rearrange("i d e -> d i e"))
    wk_p = singles.tile([D, 3, 64], BF, tag="wk_p")
    nc.sync.dma_start(out=wk_p[:], in_=aps["wk_p"].rearrange("i d e -> d i e"))
    wv = load_const("wv", [D, DC])
    wo = load_const("wo", [DC, D])
    w1 = load_const("w1", [D, DFF])
    w2 = load_const("w2", [D, 4, D])
    mask6 = load_const("mask6", [T, NH * T])
    ident = load_const("ident", [128, 128])
    if flags["qkv_bias"]:
        qb_p = load_const("qb_p", [64, 3], F32)
        kb_p = load_const("kb_p", [64, 3], F32)
        vb_rep = singles.tile([128, DC], F32, tag="vb_rep")
        nc.sync.dma_start(out=vb_rep[:], in_=aps["vb_r"].to_broadcast([128, DC]))
    if flags["bo"]:
        bo_rep = singles.tile([128, D], F32, tag="bo_rep")
        nc.sync.dma_start(out=bo_rep[:], in_=aps["bo_r"].to_broadcast([128, D]))
    if flags["b2"]:
        b2_rep = singles.tile([128, D], F32, tag="b2_rep")
        nc.sync.dma_start(out=b2_rep[:], in_=aps["b2_r"].to_broadcast([128, D]))
    if flags["b1"]:
        b1e = load_const("b1e", [128, 4], F32)

    eps_t = singles.tile([128, 1], F32, tag="eps")
    nc.vector.memset(eps_t[:], EPS)

    n_groups = bpc // G
    repeat = int(os.environ.get("K_REPEAT", "1"))
    glist = [gg for _ in range(repeat) for gg in range(n_groups)]

    def load_x(g):
        x_t = sb_g.tile([T, G, D], F32, tag="x_t")
        nc.sync.dma_start(
            out=x_t[:], in_=x_ap[g * G:(g + 1) * G].rearrange("b t d -> t b d")
        )
        return x_t

    def layer_norm(x_like, tag):
        """bn_stats/bn_aggr per batch + rstd/-mu*rstd columns for G batches."""
        st = sb_s.tile([128, G, 6], F32, tag=f"st{tag}")
        ag = sb_s.tile([128, G, 2], F32, tag=f"ag{tag}")
        for b in range(G):
            nc.vector.bn_stats(out=st[:, b, :], in_=x_like[:, b, :])
            nc.vector.bn_aggr(out=ag[:, b, :], in_=st[:, b, :])
        rstd = sb_s.tile([128, G], F32, tag=f"rstd{tag}")
        nmr = sb_s.tile([128, G], F32, tag=f"nmr{tag}")
        # rstd = exp(-0.5 * ln(var + eps)); Ln/Exp live in the same act table
        nc.scalar.activation(out=rstd[:], in_=ag[:, :, 1], func=AF.Ln,
                             bias=eps_t[:], scale=1.0)
        nc.scalar.activation(out=rstd[:], in_=rstd[:], func=AF.Exp,
                             bias=0.0, scale=-0.5)
        # nmr = -mean * rstd
        nc.vector.tensor_tensor(out=nmr[:], in0=ag[:, :, 0], in1=rstd[:],
                                op=ALU.mult)
        nc.vector.tensor_scalar_mul(out=nmr[:], in0=nmr[:], scalar1=-1.0)
        return rstd, nmr

    def phase_A(x_t):
        """LN1 + transpose + q/k/v projections for one group."""
        rstd1, nmr1 = layer_norm(x_t, "1")

        hh = sb_g.tile([T, G, D], BF, tag="hh")
        for b in range(G):
            # single-instruction centering: identity(rstd * x - mean*rstd)
            nc.scalar.activation(
                out=hh[:, b, :], in_=x_t[:, b, :], func=AF.Identity,
                bias=nmr1[:, b:b + 1], scale=rstd1[:, b:b + 1],
            )
        hhT_ps = psA.tile([D, G, T], BF, tag="Ap")
        for b in range(G):
            nc.tensor.matmul(hhT_ps[:, b, :], hh[:, b, :], ident[:],
                             is_transpose=True, start=True, stop=True,
                             skip_group_check=True)
        hhT = sb_g.tile([D, G, T], BF, tag="hhT")
        nc.vector.tensor_copy(out=hhT[:], in_=hhT_ps[:])

        qt = sb_g.tile([64, 3, G, T], BF, tag="qt")
        kt = sb_g.tile([64, 3, G, T], BF, tag="kt")
        for i in range(3):
            qk_ps = psA.tile([128, G * T], F32, tag="Ap")
            nc.tensor.matmul(
                qk_ps[0:64, :], wq_p[:, i, :],
                hhT[:].rearrange("d b t -> d (b t)"),
                start=True, stop=True, skip_group_check=True,
            )
            nc.tensor.matmul(
                qk_ps[64:128, :], wk_p[:, i, :],
                hhT[:].rearrange("d b t -> d (b t)"),
                start=True, stop=True, skip_group_check=True,
            )
            if flags["qkv_bias"]:
                nc.scalar.activation(
                    out=qt[:, i, :, :].rearrange("p b t -> p (b t)"),
                    in_=qk_ps[0:64, :],
                    func=AF.Identity, bias=qb_p[:, i:i + 1], scale=1.0,
                )
                nc.vector.tensor_scalar_add(
                    out=kt[:, i, :, :].rearrange("p b t -> p (b t)"),
                    in0=qk_ps[64:128, :], scalar1=kb_p[:, i:i + 1],
                )
            else:
                nc.scalar.copy(
                    out=qt[:, i, :, :].rearrange("p b t -> p (b t)"),
                    in_=qk_ps[0:64, :],
                )
                nc.vector.tensor_copy(
                    out=kt[:, i, :, :].rearrange("p b t -> p (b t)"),
                    in_=qk_ps[64:128, :],
                )

        v_ps = psA.tile([T, G, NH, DH], F32, tag="Ap")
        for b in range(G):
            nc.tensor.matmul(
                v_ps[:, b, :, :], hhT[:, b, :], wv[:],
                start=True, stop=True, skip_group_check=True,
            )
        v_sb = sb_g.tile([T, G, NH, DH + 1], BF, tag="v_sb")
        if flags["qkv_bias"]:
            vb3 = vb_rep[:].rearrange("p (h e) -> p h e", h=NH)
            vb4 = bass.AP(
                tensor=vb3.tensor, offset=vb3.offset,
                ap=[vb3.ap[0], [0, G], vb3.ap[1], vb3.ap[2]],
            )
            nc.vector.tensor_tensor(
                out=v_sb[:, :, :, 0:DH], in0=v_ps[:], in1=vb4, op=ALU.add,
            )
        else:
            nc.vector.tensor_copy(out=v_sb[:, :, :, 0:DH], in_=v_ps[:])
        nc.gpsimd.memset(v_sb[:, :, :, DH:DH + 1], 1.0)
        return qt, kt, v_sb

    def phase_B(x_t, qt, kt, v_sb):
        """Causal attention + residual for the G batches of one group."""
        o_pad = sb_b.tile([T, G, 128], BF, tag="o_pad")
        for b in range(G):
            sA = pss.tile([T, 384], F32, tag="sT")
            sB = pss.tile([T, 384], F32, tag="sT")
            banks = (sA, sB)
            for h in range(NH):
                i, off = h // 2, 32 * (h % 2)
                nc.tensor.matmul(
                    banks[h % 2][:, 128 * i:128 * (i + 1)],
                    kt[off:off + 32, i, b, :],
                    qt[off:off + 32, i, b, :],
                    start=True, stop=True, skip_group_check=True,
                )
            eT = sb_b.tile([T, NH * T], BF, tag="eT")
            nc.scalar.activation(out=eT[:, 0:384], in_=sA[:], func=AF.Exp)
            nc.scalar.activation(out=eT[:, 384:768], in_=sB[:], func=AF.Exp)
            # causal mask: zero the s > t entries post-exp (exp of the tiny
            # unmasked scores stays ~1, the 0/1 mask removes them from both
            # numerator and denominator); bank A on DVE (fast, unblocks its
            # o matmuls early), bank B on the otherwise-idle Pool engine
            nc.vector.tensor_tensor(out=eT[:, 0:384], in0=eT[:, 0:384],
                                    in1=mask6[:, 0:384], op=ALU.mult)
            nc.gpsimd.tensor_tensor(out=eT[:, 384:768], in0=eT[:, 384:768],
                                    in1=mask6[:, 0:384], op=ALU.mult)

            # o[t, (h, e+1)] with the softmax denominator in the last column
            o_ps = pso.tile([T, NH, DH + 1], F32, tag="op")
            for h in (0, 2, 4, 1, 3, 5):
                ecol = 384 * (h % 2) + 128 * (h // 2)
                nc.tensor.matmul(
                    o_ps[:, h, :],
                    eT[:, ecol:ecol + 128],
                    v_sb[:, b, h, :],
                    start=True, stop=True, skip_group_check=True,
                )

            recip = sb_s.tile([128, NH, 1], F32, tag="recip")
            nc.vector.reciprocal(out=recip[:], in_=o_ps[:, :, DH:DH + 1])
            nc.vector.tensor_tensor(
                out=o_pad[:, b, 0:DC].rearrange("t (h e) -> t h e", h=NH),
                in0=o_ps[:, :, 0:DH],
                in1=recip[:].to_broadcast([128, NH, DH]), op=ALU.mult,
            )

        oT_ps = pso.tile([128, G, T], BF, tag="op")
        for b in range(G):
            nc.tensor.matmul(oT_ps[:, b, :], o_pad[:, b, :], ident[:],
                             is_transpose=True, start=True, stop=True,
                             skip_group_check=True)
        oT = sb_b.tile([128, G, T], BF, tag="oT")
        nc.vector.tensor_copy(out=oT[:], in_=oT_ps[:])

        att_ps = pso.tile([T, G, D], F32, tag="op")
        for b in range(G):
            nc.tensor.matmul(att_ps[:, b, :], oT[0:DC, b, :], wo[:],
                             start=True, stop=True, skip_group_check=True)
        x1_all = sb_b.tile([T, G, D], F32, tag="x1")
        nc.vector.tensor_tensor(
            out=x1_all[:].rearrange("t b d -> t (b d)"),
            in0=x_t[:].rearrange("t b d -> t (b d)"),
            in1=att_ps[:].rearrange("t b d -> t (b d)"), op=ALU.add,
        )
        if flags["bo"]:
            bo4 = bass.AP(
                tensor=bo_rep[:].tensor, offset=bo_rep[:].offset,
                ap=[bo_rep[:].ap[0], [0, G], bo_rep[:].ap[1]],
            )
            nc.vector.tensor_tensor(out=x1_all[:], in0=x1_all[:], in1=bo4,
                                    op=ALU.add)
        return x1_all

    def phase_C(g, x1_all):
        """LN2 + feed-forward + residual + store for one group."""
        rstd2, nmr2 = layer_norm(x1_all, "2")

        h2 = sb_b.tile([T, G, D], BF, tag="h2")
        for b in range(G):
            nc.scalar.activation(
                out=h2[:, b, :], in_=x1_all[:, b, :], func=AF.Identity,
                bias=nmr2[:, b:b + 1], scale=rstd2[:, b:b + 1],
            )
        h2T_ps = psf.tile([D, G, T], BF, tag="fp")
        for b in range(G):
            nc.tensor.matmul(h2T_ps[:, b, :], h2[:, b, :], ident[:],
                             is_transpose=True, start=True, stop=True,
                             skip_group_check=True)
        h2T = sb_b.tile([D, G, T], BF, tag="h2T")
        nc.vector.tensor_copy(out=h2T[:], in_=h2T_ps[:])

        # ff1 batched over the whole group: 4 matmuls of N=G*T
        r_sb = sb_b.tile([128, 4, G, T], BF, tag="r_sb")
        for c in range(4):
            ff1_ps = psf.tile([128, G * T], F32, tag="fp")
            nc.tensor.matmul(
                ff1_ps[:], w1[:, 128 * c:128 * (c + 1)],
                h2T[:].rearrange("d b t -> d (b t)"),
                start=True, stop=True, skip_group_check=True,
            )
            if flags["b1"]:
                nc.vector.tensor_scalar(
                    out=r_sb[:, c, :, :].rearrange("p b t -> p (b t)"),
                    in0=ff1_ps[:], scalar1=b1e[:, c:c + 1], scalar2=0.0,
                    op0=ALU.add, op1=ALU.max,
                )
            else:
                nc.vector.tensor_scalar_max(
                    out=r_sb[:, c, :, :].rearrange("p b t -> p (b t)"),
                    in0=ff1_ps[:], scalar1=0.0,
                )

        out_sb = sb_b.tile([T, G, D], F32, tag="out_sb")
        for p in range(G // 2):
            ff2_ps = pso.tile([T, 2, D], F32, tag="op")
            for j in range(2):
                b = 2 * p + j
                for c in range(4):
                    nc.tensor.matmul(
                        ff2_ps[:, j, :], r_sb[:, c, b, :], w2[:, c, :],
                        start=(c == 0), stop=(c == 3), skip_group_check=True,
                    )
            nc.vector.tensor_tensor(
                out=out_sb[:, 2 * p:2 * p + 2, :].rearrange("t b d -> t (b d)"),
                in0=x1_all[:, 2 * p:2 * p + 2, :].rearrange("t b d -> t (b d)"),
                in1=ff2_ps[:].rearrange("t b d -> t (b d)"), op=ALU.add,
            )
        if flags["b2"]:
            b24 = bass.AP(
                tensor=b2_rep[:].tensor, offset=b2_rep[:].offset,
                ap=[b2_rep[:].ap[0], [0, G], b2_rep[:].ap[1]],
            )
            nc.vector.tensor_tensor(out=out_sb[:], in0=out_sb[:], in1=b24,
                                    op=ALU.add)
        nc.gpsimd.dma_start(
            out=y_ap[g * G:(g + 1) * G].rearrange("b t d -> t b d"),
            in_=out_sb[:],
        )

    # software pipeline: B(g) | A(g+1) | C(g)
    x_cur = load_x(glist[0])
    A_cur = phase_A(x_cur)
    for gi, g in enumerate(glist):
        more = gi + 1 < len(glist)
        if more:
            x_nxt = load_x(glist[gi + 1])
        x1 = phase_B(x_cur, *A_cur)
        if more:
            A_nxt = phase_A(x_nxt)
        phase_C(g, x1)
        if more:
            x_cur, A_cur = x_nxt, A_nxt


def build_program(weights, flags, bpc=BPC):
    nc = _Bacc("TRN2", target_bir_lowering=False, debug=False)
    aps = {}
    aps["x"] = nc.dram_tensor("x", [bpc, T, D], F32, kind="ExternalInput").ap()
    aps["y"] = nc.dram_tensor("y", [bpc, T, D], F32, kind="ExternalOutput").ap()
    for name, arr in weights.items():
        dt = F32 if arr.dtype == np.float32 else BF
        aps[name] = nc.dram_tensor(name, list(arr.shape), dt, kind="ExternalInput").ap()
    with tile.TileContext(nc) as tc:
        with ExitStack() as ctx:
            _emit(ctx, tc, aps, flags, bpc)
    nc.compile()
    return nc


_CACHE = {}


def _get_program_and_maps(x, args):
    x = np.asarray(x, np.float32)
    weights, flags = _prep_weights(*args)
    key = tuple(sorted(flags.items()))
    if key not in _CACHE:
        _CACHE[key] = build_program(weights, flags)
    nc = _CACHE[key]
    in_maps = []
    for c in range(NCORES):
        m = {"x": np.ascontiguousarray(x[c * BPC:(c + 1) * BPC])}
        m.update(weights)
        in_maps.append(m)
    return nc, in_maps


def kernel(x, Wq, Wk, Wv, Wo, bo, W1, b1, W2, b2, g1, beta1, g2, beta2):
    nc, in_maps = _get_program_and_maps(
        x, (Wq, Wk, Wv, Wo, bo, W1, b1, W2, b2, g1, beta1, g2, beta2)
    )
    res = run_bass_kernel_spmd(nc, in_maps, list(range(NCORES)))
    out = np.concatenate([res.results[c]["y"] for c in range(NCORES)], axis=0)
    return out.astype(np.float32)


def run_traced(inputs):
    """Profiled run; returns BassKernelResults with exec_time_ns if available."""
    args = tuple(
        inputs[k]
        for k in ("Wq", "Wk", "Wv", "Wo", "bo", "W1", "b1", "W2", "b2",
                  "g1", "beta1", "g2", "beta2")
    )
    nc, in_maps = _get_program_and_maps(inputs["x"], args)
    return run_bass_kernel_spmd(nc, in_maps, list(range(NCORES)), trace=True)
